# revision 19
# baseline (speedup 1.0000x reference)
"""Trainium2 Bass kernel for nn_DecoderA (neural BP / GNN message passing decoder).

Strategy: pure data parallel over batch (128 items -> 8 cores x 16 items).
Per core, 16 items as 4 groups of 4; the (b,m)-rows of each group form 9
tiles of [128, 576].  All 4 groups' message state M, weights Wg and mask
live in SBUF in fp16 for the full 5 BP iterations; iterations are the
outer loop so all 36 tiles of one iteration pipeline together.

Per iteration t, per tile j (pairs of tiles share wide tail ops):

  PSUM   vr   = -I@M + Esel@A          (fp16 PE matmuls; A = x_t + sum_m M)
  DVE    v2   = (vr - 40) * mask       (stt from PSUM; off-mask -> 0)
  ACT    te   = tanh(0.5*v2 + 20)      (== masked tanh(V/2), 1 off-mask;
                                        no clip needed: tanh saturation +
                                        the ln route reproduce clip(+-15)
                                        to ~1e-6)
  DVE    P    = prod_n te              (tensor_reduce mult)
  ACT    a1   = |te + P|               (Abs, bias=P)
  DVE    a2   = max(|te - P|, 1e-38)   (tensor_scalar, AP scalar + abs_max)
  ACT    l1   = ln(a1 + 1e-38)         (pair-wide)
  ACT    l2   = ln(a2 + 1e-38)         (pair-wide, in place over a2)
  GPS    lq   = l1 - l2                (== 2*atanh(P/te), division-free)
  DVE    lqc  = clip(lq, +-2*atanh(1-1e-6))
  GPS    d    = lqc * Wg               (Wg = gate*w_cv*H, fp16 from host)
  DVE    M    = M*(1-gate) + d         (stt, fp16 out)
  PE     post = Esel^T @ M             (16-item-wide accumulation over all
                                        36 tiles of the iteration)

The posterior PSUM is [16, .] (one partition per item, all groups) so a
single accumulation chain and a single a_{t+1} = post + x_{t+1} add serve
every group; posteriors DMA straight from PSUM to DRAM per iteration.
Host does the cheap pre/post work (LLR normalization, pooling, sigmoid).
"""

import sys

import numpy as np

sys.path.insert(0, "/opt/trn_rl_repo")

import concourse.bacc as bacc  # noqa: E402
import concourse.tile as tile  # noqa: E402
from concourse import mybir  # noqa: E402
from concourse.bass_utils import run_bass_kernel_spmd  # noqa: E402

F32 = mybir.dt.float32
F16 = mybir.dt.float16
ALU = mybir.AluOpType
ACT = mybir.ActivationFunctionType

B = 128
MCHK = 288
NVAR = 576
KINFO = 288
T = 5
NCORES = 8
BL = B // NCORES          # 16 items per core
GI = 4                    # items per group
NG = BL // GI             # 4 groups
NT = GI * MCHK // 128     # 9 tiles of [128, NVAR] per group
HC = NVAR // 2            # 288, matmul N-chunk (<=512 per PSUM bank)

_CLIP_C = float(2.0 * np.arctanh(np.float64(np.float32(1.0 - 1e-6))))


def _build(gate: float):
    nc = bacc.Bacc("TRN2", target_bir_lowering=False, debug=False)

    wg_d = nc.dram_tensor("wg", [BL * MCHK, NVAR], F16, kind="ExternalInput").ap()
    mk_d = nc.dram_tensor("mk", [BL * MCHK, NVAR], F16, kind="ExternalInput").ap()
    xs_d = nc.dram_tensor("xs", [BL, T * NVAR], F16, kind="ExternalInput").ap()
    esw_d = nc.dram_tensor("esw", [128, NG * NT * BL], F16, kind="ExternalInput").ap()
    est_d = nc.dram_tensor("est", [BL, NG * NT * 128], F16, kind="ExternalInput").ap()
    negi_d = nc.dram_tensor("negi", [128, 128], F16, kind="ExternalInput").ap()
    posts_d = nc.dram_tensor("posts", [BL, T * NVAR], F32, kind="ExternalOutput").ap()

    one_m_g = float(1.0 - gate)

    with tile.TileContext(nc) as tc:
        with (
            tc.tile_pool(name="consts", bufs=1) as consts,
            tc.tile_pool(name="wg", bufs=1) as wg_pool,
            tc.tile_pool(name="mask", bufs=1) as mask_pool,
            tc.tile_pool(name="mstate", bufs=1) as m_pool,
            tc.tile_pool(name="acur", bufs=2) as a_pool,
            tc.tile_pool(name="te", bufs=4) as te_pool,
            tc.tile_pool(name="a1", bufs=4) as a1_pool,
            tc.tile_pool(name="lq", bufs=4) as lq_pool,
            tc.tile_pool(name="pt", bufs=6) as pt_pool,
            tc.tile_pool(name="psum_v", bufs=2, space="PSUM") as psv_pool,
            tc.tile_pool(name="psum_post", bufs=2, space="PSUM") as psp_pool,
        ):
            eselw = consts.tile([128, NG, NT, BL], F16)
            nc.sync.dma_start(
                out=eselw, in_=esw_d.rearrange("p (g j k) -> p g j k", g=NG, j=NT)
            )
            eseltw = consts.tile([BL, NG, NT, 128], F16)
            nc.sync.dma_start(
                out=eseltw, in_=est_d.rearrange("k (g j p) -> k g j p", g=NG, j=NT)
            )
            negi = consts.tile([128, 128], F16)
            nc.sync.dma_start(out=negi, in_=negi_d)
            b0 = consts.tile([128, 1], F32)
            nc.vector.memset(b0, 0.0)
            b38 = consts.tile([128, 1], F32)
            nc.vector.memset(b38, 1e-38)
            xsall = consts.tile([BL, T, 2, HC], F16)
            nc.sync.dma_start(
                out=xsall, in_=xs_d.rearrange("b (t c n) -> b t c n", t=T, c=2)
            )
            postsall = consts.tile([BL, T, 2, HC], F32)

            wg_all = wg_pool.tile([128, NG, NT, NVAR], F16)
            mk_all = mask_pool.tile([128, NG, NT, NVAR], F16)
            m_all = m_pool.tile([128, NG, NT, NVAR], F16)
            for g in range(NG):
                nc.sync.dma_start(
                    out=wg_all[:, g],
                    in_=wg_d[g * NT * 128 : (g + 1) * NT * 128, :].rearrange(
                        "(j p) n -> p j n", p=128
                    ),
                )
                nc.sync.dma_start(
                    out=mk_all[:, g],
                    in_=mk_d[g * NT * 128 : (g + 1) * NT * 128, :].rearrange(
                        "(j p) n -> p j n", p=128
                    ),
                )

            posts_v = posts_d.rearrange("b (t c n) -> b t c n", t=T, c=2)
            pairs = [(0, 1), (2, 3), (4, 5), (6, 7), (8,)]

            a_all = a_pool.tile([BL, 2, HC], F16, tag="acur", name="acur")
            nc.vector.tensor_copy(a_all, xsall[:, 0])
            for t in range(T):
                post_ps = psp_pool.tile([BL, 2, 512], F32, tag="post")
                for pi, pj in enumerate(pairs):
                    w = len(pj)
                    j0 = pj[0]
                    for g in range(NG):
                        tep = te_pool.tile([128, 2, NVAR], F32, tag="te",
                                           name="te")[:, :w]
                        a1p = a1_pool.tile([128, 2, NVAR], F32, tag="a1",
                                           name="a1")[:, :w]
                        p2 = pt_pool.tile([128, 2], F32, tag="pt", name="pt")
                        for jj, j in enumerate(pj):
                            v_ps = psv_pool.tile([128, 2, 512], F32, tag="v")
                            for c in range(2):
                                if t > 0:
                                    nc.tensor.matmul(
                                        v_ps[:, c, :HC], negi,
                                        m_all[:, g, j, c * HC : (c + 1) * HC],
                                        start=True, stop=False,
                                    )
                                # -I @ mk adds +192 on off-mask lanes so tanh
                                # saturates them to exactly 1
                                nc.tensor.matmul(
                                    v_ps[:, c, :HC], negi,
                                    mk_all[:, g, j, c * HC : (c + 1) * HC],
                                    start=(t == 0), stop=False,
                                )
                                nc.tensor.matmul(
                                    v_ps[:, c, :HC], eseltw[:, g, j], a_all[:, c],
                                    start=False, stop=True,
                                )
                            # te = tanh(0.5*V) straight from PSUM (1 off-mask)
                            nc.scalar.activation(
                                tep[:, jj].rearrange("p (c n) -> p c n", c=2),
                                v_ps[:, :, :HC], ACT.Tanh, bias=b0, scale=0.5,
                            )
                        # P = prod_n te, both tiles in one reduce
                        nc.vector.tensor_reduce(
                            out=p2[:, :w], in_=tep,
                            axis=mybir.AxisListType.X, op=ALU.mult,
                        )
                        # P -> P/2 shrunk so |P| < |te| strictly: keeps
                        # s2 = (te - P)/2 away from +-0 (approx reciprocal
                        # is undefined there)
                        nc.vector.tensor_scalar(
                            out=p2, in0=p2, scalar1=float(0.5 * (1.0 - 1e-7)),
                            scalar2=None, op0=ALU.mult,
                        )
                        for jj, j in enumerate(pj):
                            # s1 = (te + P)/2; |P| < |te| so s1/s2 > 0 and
                            # ln(s1/s2) = 2*atanh(P/te) with no abs needed
                            nc.scalar.activation(
                                a1p[:, jj], tep[:, jj], ACT.Identity,
                                bias=p2[:, jj : jj + 1], scale=0.5,
                            )
                        # s2 = (te - P)/2 == te - s1  (plain tensor_tensor)
                        nc.vector.tensor_tensor(out=tep, in0=tep, in1=a1p,
                                                op=ALU.subtract)
                        # ---- pair-wide tail ----
                        nc.vector.reciprocal_approx_fast(tep, tep)
                        nc.gpsimd.tensor_tensor(out=tep, in0=a1p, in1=tep,
                                                op=ALU.mult)
                        nc.scalar.activation(tep, tep, ACT.Ln, bias=b38)
                        lqf = lq_pool.tile([128, 2, NVAR], F16, tag="lq",
                                           name="lq")[:, :w]
                        nc.vector.tensor_scalar(
                            out=lqf, in0=tep, scalar1=_CLIP_C, scalar2=-_CLIP_C,
                            op0=ALU.min, op1=ALU.max,
                        )
                        mpair = m_all[:, g, j0 : j0 + w]
                        wgp = wg_all[:, g, j0 : j0 + w]
                        if t == 0:
                            nc.gpsimd.tensor_tensor(out=mpair, in0=lqf, in1=wgp,
                                                    op=ALU.mult)
                        else:
                            nc.gpsimd.tensor_tensor(out=lqf, in0=lqf, in1=wgp,
                                                    op=ALU.mult)
                            nc.vector.scalar_tensor_tensor(
                                out=mpair, in0=mpair, scalar=one_m_g, in1=lqf,
                                op0=ALU.mult, op1=ALU.add,
                            )
                        # posterior accumulation: post += Esel_j^T @ M_j
                        first = pi == 0 and g == 0
                        last = pi == len(pairs) - 1 and g == NG - 1
                        for jj, j in enumerate(pj):
                            for c in range(2):
                                nc.tensor.matmul(
                                    post_ps[:, c, :HC],
                                    eselw[:, g, j],
                                    m_all[:, g, j, c * HC : (c + 1) * HC],
                                    start=(first and jj == 0),
                                    stop=(last and jj == w - 1),
                                )
                # posts_raw[:, t] = post (host adds x_t)
                nc.scalar.copy(postsall[:, t], post_ps[:, :, :HC])
                nc.sync.dma_start(out=posts_v[:, t], in_=postsall[:, t])
                if t + 1 < T:
                    a_all = a_pool.tile([BL, 2, HC], F16, tag="acur", name="acur")
                    nc.vector.tensor_add(a_all, post_ps[:, :, :HC], xsall[:, t + 1])
    nc.compile()
    return nc


_CACHE = {}


def _get_nc(gate: float):
    key = round(gate, 12)
    if key not in _CACHE:
        _CACHE[key] = _build(gate)
    return _CACHE[key]


def _host_prep(inputs, H, sigma2, input_ponderation, w_cv, gate_logit):
    f32 = np.float32
    f16 = np.float16
    gate = float(1.0 / (1.0 + np.exp(-np.float64(gate_logit))))

    llrs = (f32(-4.0) * inputs / sigma2).astype(f32)
    norm_llrs = llrs / np.mean(np.abs(llrs), axis=-1, keepdims=True, dtype=f32)
    xs = (norm_llrs[:, None, :] * input_ponderation[None, :, :]).astype(f32)  # [B,T,N]

    wg_full = (f32(gate) * w_cv[None, :, :] * H.astype(f32)).astype(f32)  # [B,M,N]

    # selector constants (same for every core): row p of tile (g,j) belongs to
    # item slot 4*g + (j*128+p)//MCHK of the 16-item posterior/a tiles
    eselw = np.zeros((128, NG, NT, BL), f16)
    eseltw = np.zeros((BL, NG, NT, 128), f16)
    for j in range(NT):
        for p in range(128):
            k = (j * 128 + p) // MCHK
            for g in range(NG):
                eselw[p, g, j, GI * g + k] = 1.0
                eseltw[GI * g + k, g, j, p] = 1.0
    negi = (-np.eye(128)).astype(f16)
    esw = np.ascontiguousarray(eselw.reshape(128, NG * NT * BL))
    est = np.ascontiguousarray(eseltw.reshape(BL, NG * NT * 128))

    in_maps = []
    for c in range(NCORES):
        sl = slice(c * BL, (c + 1) * BL)
        in_maps.append(
            {
                "wg": np.ascontiguousarray(
                    wg_full[sl].reshape(BL * MCHK, NVAR).astype(f16)
                ),
                "mk": np.ascontiguousarray(
                    ((H[sl].astype(f32) - f32(1.0)) * f32(192.0))
                    .reshape(BL * MCHK, NVAR).astype(f16)
                ),
                "xs": np.ascontiguousarray(
                    xs[sl].reshape(BL, T * NVAR).astype(f16)
                ),
                "esw": esw,
                "est": est,
                "negi": negi,
            }
        )
    return gate, norm_llrs, xs, in_maps


def _host_post(posts_raw, xs, norm_llrs, out_ponderation, skip_ponderation):
    f32 = np.float32
    posts = (posts_raw + xs).astype(f32)  # add x_t back in
    norm_out = posts / np.mean(np.abs(posts), axis=-1, keepdims=True, dtype=f32)
    pooled = np.mean(out_ponderation[None] * norm_out, axis=-2, dtype=f32)
    out = (pooled + skip_ponderation * norm_llrs).astype(f32)
    return (1.0 / (1.0 + np.exp(out[:, :KINFO], dtype=f32))).astype(f32)


def run(trace=False, **inputs):
    inputs = {k: np.asarray(v) for k, v in inputs.items()}
    gate, norm_llrs, xs, in_maps = _host_prep(
        inputs["inputs"],
        inputs["H"],
        inputs["sigma2"],
        inputs["input_ponderation"],
        inputs["w_cv"],
        inputs["gate_logit"],
    )
    nc = _get_nc(gate)
    res = run_bass_kernel_spmd(
        nc, in_maps, core_ids=list(range(NCORES)), trace=trace
    )
    posts_raw = np.concatenate(
        [r["posts"].reshape(BL, T, NVAR) for r in res.results], axis=0
    )
    out = _host_post(
        posts_raw, xs, norm_llrs,
        inputs["out_ponderation"], inputs["skip_ponderation"],
    )
    return out, res


def kernel(**inputs) -> np.ndarray:
    out, _ = run(trace=False, **inputs)
    return out
